# revision 4
# baseline (speedup 1.0000x reference)
"""VQ Euclidean-codebook kernel for Trainium2 (8 NeuronCores, data-parallel).

Per core (N_loc = 16384 rows of x):
  scores s[n,k] = x[n]·e[k] - 0.5*||e[k]||^2   (argmax_k s  ==  argmin_k ||x-e||^2)
  - PE: per 128-row tile, transpose x tile (identity matmul) then 8 matmuls
    of [65-free chunks] xT.T @ embT into PSUM (fp32).
  - ACT: copies PSUM score chunks into an SBUF scratch row-block.
  - DVE: one custom single-pass scan op computes argmax index (value stream =
    in0 + in1 where in1 carries the -0.5*||e||^2 broadcast) -> accum_out.
  - GPSIMD: indirect DMA gathers embed[idx] rows; HWDGE stores them to DRAM.

The host shards x row-wise across 8 cores, replicates embed, and concatenates
the per-core outputs.
"""

import numpy as np

import concourse.bass as bass
import concourse.bacc as bacc
import concourse.mybir as mybir
from concourse.tile import TileContext
from concourse.bass_utils import run_bass_kernel_spmd
from concourse.masks import make_identity

from concourse import dve_ops
from concourse.dve_spec import (
    Spec, Src0, Src1, Zero, One, AluOp, Idx, scan, select, eq, lower,
)
from concourse.dve_uop import DveOpSpec

P = 128          # partitions / rows per tile
N_FULL = 131072  # total rows
N_CORES = 8
N_LOC = N_FULL // N_CORES  # 16384
K = 4096         # codebook size
D = 64           # feature dim
NT = N_LOC // P  # 128 tiles per core
F32 = mybir.dt.float32

_ARGMAX_OP_NAME = "ARGMAX_ADD_SCAN_ANT"


def _argmax_add_reference(in0, in1, c0, c1, c2):
    v = (np.asarray(in0, np.float32) + np.asarray(in1, np.float32)).astype(np.float32)
    v2 = v.reshape(v.shape[0], -1)
    r = np.maximum.accumulate(v2, axis=1)
    qual = v2 == r
    idxs = np.arange(v2.shape[1], dtype=np.float32)[None, :]
    body = np.where(qual, idxs, -1.0).astype(np.float32)
    acc = body.max(axis=1, keepdims=True)
    return body.reshape(in0.shape), acc


def register_argmax_op():
    """Register the fused argmax-of-(in0+in1) scan op with the DVE table.

    out[p,k]    = k if (in0+in1)[p,k] == running-max else -1
    accum_out[p] = max_k out[p,k]  == index of the (last) maximum.
    """
    for op in dve_ops.OPS:
        if op.name == _ARGMAX_OP_NAME:
            return op
    v = Src0 + Src1
    body = select(eq(v, scan(AluOp.MAX, v)), Idx, Zero - One)
    spec = Spec(body=body, accum=AluOp.MAX, reference=_argmax_add_reference)
    row = dve_ops._CUSTOM_DVE_ROW_BASE + len(dve_ops.OPS)
    dve_ops._SUB_OPCODE_FOR_NAME[_ARGMAX_OP_NAME] = row
    uops = lower(spec, ver="v3")
    sha = DveOpSpec(name=_ARGMAX_OP_NAME, opcode=row, uops=uops, rd1_en=True).sha("v3")
    op = dve_ops.DveOp(_ARGMAX_OP_NAME, spec, subdim=False, uops_sha={"v3": sha})
    dve_ops.OPS.append(op)
    dve_ops.CUSTOM_DVE_SPECS[_ARGMAX_OP_NAME] = spec
    return op


def build(r_iters: int = 1, score_bufs: int = 2):
    """Trace + compile the per-core program. r_iters>1 wraps the main loop in a
    hardware For loop (identical passes) for wall-clock timing experiments."""
    argmax_op = register_argmax_op()
    nc = bacc.Bacc(num_devices=N_CORES)
    x_in = nc.dram_tensor("x", [N_LOC, D], F32, kind="ExternalInput")
    emb_in = nc.dram_tensor("embed", [K, D], F32, kind="ExternalInput")
    q_out = nc.dram_tensor("q", [N_LOC, D], F32, kind="ExternalOutput")

    # score-chunk sizes per n-tile: 3 PSUM tiles (<=3 banks each), 8 matmuls
    CHUNKS = [(0, 1536), (1536, 1536), (3072, 1024)]

    with TileContext(nc) as tc:
        with (
            tc.tile_pool(name="const", bufs=1) as cpool,
            tc.tile_pool(name="xload", bufs=4) as xpool,
            tc.tile_pool(name="xt", bufs=4) as xtpool,
            tc.tile_pool(name="score", bufs=score_bufs) as spool,
            tc.tile_pool(name="junk", bufs=1) as jpool,
            tc.tile_pool(name="idx", bufs=4) as ipool,
            tc.tile_pool(name="gather", bufs=4) as gpool,
            tc.tile_pool(name="pscore", bufs=2, space="PSUM") as pspool,
            tc.tile_pool(name="ptrans", bufs=2, space="PSUM") as ptpool,
        ):
            # ---------------- setup ----------------
            ident = cpool.tile([P, P], F32)
            make_identity(nc, ident[:, :])

            # embT[d, k] = embed[k, d] via 32 PE transposes
            embT = cpool.tile([D, K], F32)
            for t in range(K // P):
                et = xpool.tile([P, D], F32, tag="eload")
                nc.sync.dma_start(out=et[:, :], in_=emb_in[t * P:(t + 1) * P, :])
                pt = ptpool.tile([D, P], F32, tag="pt")
                nc.tensor.transpose(pt[:, :], et[:, :], ident[:, :])
                nc.scalar.copy(out=embT[:, t * P:(t + 1) * P], in_=pt[:, :])

            # e2b[p, k] = -0.5 * sum_d embed[k,d]^2, replicated on all partitions
            sq = cpool.tile([D, K], F32)
            nc.vector.tensor_mul(sq[:, :], embT[:, :], embT[:, :])
            negh = cpool.tile([D, P], F32)
            nc.vector.memset(negh[:, :], -0.5)
            e2b = cpool.tile([P, K], F32)
            for c in range(K // 512):
                pe2 = pspool.tile([P, 512], F32, tag="ps")
                nc.tensor.matmul(out=pe2[:, :], lhsT=negh[:, :],
                                 rhs=sq[:, c * 512:(c + 1) * 512],
                                 start=True, stop=True)
                nc.scalar.copy(out=e2b[:, c * 512:(c + 1) * 512], in_=pe2[:, :])

            # ---------------- main loop ----------------
            def tile_body(t):
                xt = xpool.tile([P, D], F32)
                nc.sync.dma_start(out=xt[:, :], in_=x_in[t * P:(t + 1) * P, :])
                ptr = ptpool.tile([D, P], F32, tag="pt")
                nc.tensor.transpose(ptr[:, :], xt[:, :], ident[:, :])
                xT = xtpool.tile([D, P], F32)
                nc.scalar.copy(out=xT[:, :], in_=ptr[:, :])

                sc = spool.tile([P, K], F32)
                for (off, width) in CHUNKS:
                    ps = pspool.tile([P, 1536], F32, tag="ps")
                    for c in range(width // 512):
                        nc.tensor.matmul(
                            out=ps[:, c * 512:(c + 1) * 512],
                            lhsT=xT[:, :],
                            rhs=embT[:, off + c * 512: off + (c + 1) * 512],
                            start=True, stop=True,
                        )
                    nc.scalar.copy(out=sc[:, off:off + width], in_=ps[:, :width])

                junk = jpool.tile([P, K], F32)
                idxf = ipool.tile([P, 1], F32)
                nc.vector._custom_dve(
                    argmax_op, out=junk[:, :], in0=sc[:, :], in1=e2b[:, :],
                    accum_out=idxf[:, :],
                )
                idxi = ipool.tile([P, 1], mybir.dt.int32)
                nc.vector.tensor_copy(out=idxi[:, :], in_=idxf[:, :])

                g = gpool.tile([P, D], F32)
                nc.gpsimd.indirect_dma_start(
                    out=g[:, :], out_offset=None, in_=emb_in[:, :],
                    in_offset=bass.IndirectOffsetOnAxis(ap=idxi[:, :1], axis=0),
                )
                nc.sync.dma_start(out=q_out[t * P:(t + 1) * P, :], in_=g[:, :])

            if r_iters == 1:
                for t in range(NT):
                    tile_body(t)
            else:
                with tc.For_i(0, r_iters, 1):
                    for t in range(NT):
                        tile_body(t)

    nc.compile()
    return nc


_CACHED_NC = None


def kernel(x: np.ndarray, embed: np.ndarray) -> np.ndarray:
    global _CACHED_NC
    assert x.shape == (N_FULL, D) and embed.shape == (K, D), (
        f"hardcoded for x[{N_FULL},{D}], embed[{K},{D}]; got {x.shape}, {embed.shape}"
    )
    x = np.ascontiguousarray(x, dtype=np.float32)
    embed = np.ascontiguousarray(embed, dtype=np.float32)
    if _CACHED_NC is None:
        _CACHED_NC = build()
    nc = _CACHED_NC
    shards = np.split(x, N_CORES, axis=0)
    in_maps = [{"x": s, "embed": embed} for s in shards]
    res = run_bass_kernel_spmd(nc, in_maps, core_ids=list(range(N_CORES)))
    return np.concatenate([r["q"] for r in res.results], axis=0)


# revision 11
# speedup vs baseline: 16.9876x; 16.9876x over previous
"""VQ Euclidean-codebook kernel for Trainium2 (8 NeuronCores, data-parallel).

Math: quantize[n] = embed[argmin_k ||x[n]-embed[k]||^2]
    = embed[argmax_k (x[n]·embed[k] - 0.5*||embed[k]||^2)]

Per core (N_loc = 16384 rows of x, codebook replicated):
  - Host marshals the core's x shard transposed (xT [64, N_loc]) and embed
    transposed (embT [64, K]); both are pure layout changes of the inputs.
  - PE: per 128-row tile, 8 fp32 matmuls xT_tile.T @ embT chunks -> PSUM
    (exact fp32; argmin gaps in this problem are ~4e-5, far above fp32
    rounding, so the argmax is exact).
  - ACT: copies PSUM score chunks ([128, 2048] x2) into an SBUF scratch.
  - DVE: ONE custom single-pass scan op per tile computes the argmax index:
    value stream v = in0 + in1 (in1 = -0.5*||e||^2 broadcast), running-max
    scan, qual = (v == runmax), out = select(qual, Idx, -1), accum = MAX ->
    index of the row maximum, directly in accum_out. This is the minimal
    one-read-per-element argmax on the DVE.
  - GPSIMD: indirect DMA gathers embed[idx] rows; HWDGE stores to DRAM.

Measured bottleneck is the PE fp32 matmul (4 cycles/row: 2 weight passes x
2-cycle fp32 moving-operand feed); DVE scan, ACT copies, gathers and all DMA
overlap underneath it.
"""

import numpy as np

import concourse.bass as bass
import concourse.bacc as bacc
import concourse.mybir as mybir
from concourse.tile import TileContext
from concourse.bass_utils import run_bass_kernel_spmd

from concourse import dve_ops
from concourse.dve_spec import (
    Spec, Src0, Src1, Zero, One, AluOp, Idx, scan, select, eq, lower,
)
from concourse.dve_uop import DveOpSpec

P = 128          # partitions / rows per tile
N_FULL = 131072  # total rows
N_CORES = 8
N_LOC = N_FULL // N_CORES  # 16384
K = 4096         # codebook size
D = 64           # feature dim
NT = N_LOC // P  # 128 tiles per core
F32 = mybir.dt.float32

_ARGMAX_OP_NAME = "ARGMAX_ADD_SCAN_ANT"


def _argmax_add_reference(in0, in1, c0, c1, c2):
    v = (np.asarray(in0, np.float32) + np.asarray(in1, np.float32)).astype(np.float32)
    v2 = v.reshape(v.shape[0], -1)
    r = np.maximum.accumulate(v2, axis=1)
    qual = v2 == r
    idxs = np.arange(v2.shape[1], dtype=np.float32)[None, :]
    body = np.where(qual, idxs, -1.0).astype(np.float32)
    acc = body.max(axis=1, keepdims=True)
    return body.reshape(in0.shape), acc


def register_argmax_op():
    """Register the fused argmax-of-(in0+in1) scan op in the DVE table."""
    for op in dve_ops.OPS:
        if op.name == _ARGMAX_OP_NAME:
            return op
    v = Src0 + Src1
    body = select(eq(v, scan(AluOp.MAX, v)), Idx, Zero - One)
    spec = Spec(body=body, accum=AluOp.MAX, reference=_argmax_add_reference)
    row = dve_ops._CUSTOM_DVE_ROW_BASE + len(dve_ops.OPS)
    dve_ops._SUB_OPCODE_FOR_NAME[_ARGMAX_OP_NAME] = row
    uops = lower(spec, ver="v3")
    sha = DveOpSpec(name=_ARGMAX_OP_NAME, opcode=row, uops=uops, rd1_en=True).sha("v3")
    op = dve_ops.DveOp(_ARGMAX_OP_NAME, spec, subdim=False, uops_sha={"v3": sha})
    dve_ops.OPS.append(op)
    dve_ops.CUSTOM_DVE_SPECS[_ARGMAX_OP_NAME] = spec
    return op


def build(r_iters: int = 1, score_bufs: int = 3, fp16_split: bool = True,
          mm_width: int = 512):
    """Trace + compile the per-core program.

    fp16_split: compute xe with an exact-in-practice two-term fp16 split of
    both operands (x=x1+x2, e=e1+e2; s = x1e1+x1e2+x2e1, dropping the
    O(2^-24)-relative x2e2 term). Three 1-cycle/row fp16 matmuls beat one
    4-cycle/row fp32 matmul; residual error ~1e-6, two orders below the
    smallest argmin gap in this problem. PE honors fp16 denormals (verified
    on HW), which the x2/e2 terms need.

    r_iters>1 wraps the main loop in a hardware For loop for timing.
    """
    argmax_op = register_argmax_op()
    F16 = mybir.dt.float16
    nc = bacc.Bacc(num_devices=N_CORES)
    xT_in = nc.dram_tensor("xT", [D, N_LOC], F32, kind="ExternalInput")
    embT_in = nc.dram_tensor("embT", [D, K], F32, kind="ExternalInput")
    emb_in = nc.dram_tensor("embed", [K, D], F32, kind="ExternalInput")
    q_out = nc.dram_tensor("q", [N_LOC, D], F32, kind="ExternalOutput")

    KH = 2048  # score chunk per PSUM tile (4 banks); 2 chunks per n-tile

    with TileContext(nc) as tc:
        with (
            tc.tile_pool(name="const", bufs=1) as cpool,
            tc.tile_pool(name="score", bufs=score_bufs) as spool,
            tc.tile_pool(name="junk", bufs=1) as jpool,
            tc.tile_pool(name="idx", bufs=4) as ipool,
            tc.tile_pool(name="gather", bufs=4) as gpool,
            tc.tile_pool(name="pscore", bufs=2, space="PSUM") as pspool,
        ):
            # ---------------- setup ----------------
            negh = cpool.tile([D, P], F32)
            nc.vector.memset(negh[:, :], -0.5)
            e2b = cpool.tile([P, K], F32)

            if fp16_split:
                # xs = [x1; x2] stacked on partitions (x1 = fp16(x),
                # x2 = fp16(x - x1)); e1d/e2d are the embed halves duplicated
                # across both partition halves. Per k-chunk the scores are two
                # C=128 fp16 matmuls: xs.T@[e1;e1] = x1e1+x2e1 and
                # xs.T@[e2;e2] = x1e2+x2e2 — together the full
                # (x1+x2)(e1+e2) with fp32 PSUM accumulation.
                xs = cpool.tile([2 * D, N_LOC], F16)
                e1d = cpool.tile([2 * D, K], F16)
                e2d = cpool.tile([2 * D, K], F16)
                with tc.tile_pool(name="stage", bufs=3) as stpool:
                    SW = 2048
                    for c in range(N_LOC // SW):
                        sl = slice(c * SW, (c + 1) * SW)
                        st = stpool.tile([2 * D, SW], F32, tag="st")
                        nc.sync.dma_start(out=st[0:D, :], in_=xT_in[:, sl])
                        nc.sync.dma_start(out=st[D:2 * D, :], in_=xT_in[:, sl])
                        tmp = stpool.tile([2 * D, SW], F16, tag="tmp")
                        nc.vector.tensor_copy(out=tmp[:, :], in_=st[:, :])
                        nc.vector.tensor_copy(out=xs[0:D, sl], in_=tmp[0:D, :])
                        nc.vector.tensor_sub(
                            xs[D:2 * D, sl], st[D:2 * D, :], tmp[D:2 * D, :])
                    for c in range(K // SW):
                        sl = slice(c * SW, (c + 1) * SW)
                        st = stpool.tile([2 * D, SW], F32, tag="st")
                        nc.sync.dma_start(out=st[0:D, :], in_=embT_in[:, sl])
                        nc.sync.dma_start(out=st[D:2 * D, :], in_=embT_in[:, sl])
                        tmp = stpool.tile([2 * D, SW], F16, tag="tmp")
                        nc.vector.tensor_copy(out=tmp[:, :], in_=st[:, :])
                        nc.vector.tensor_copy(out=e1d[:, sl], in_=tmp[:, :])
                        nc.vector.tensor_sub(e2d[:, sl], st[:, :], tmp[:, :])
                        # sq chunk in place, then -0.5*colsum via ones matmul
                        nc.vector.tensor_mul(st[0:D, :], st[0:D, :], st[0:D, :])
                        for q in range(SW // 512):
                            pe2 = pspool.tile([P, 512], F32, tag="ps")
                            nc.tensor.matmul(
                                out=pe2[:, :], lhsT=negh[:, :],
                                rhs=st[0:D, q * 512:(q + 1) * 512],
                                start=True, stop=True)
                            nc.scalar.copy(
                                out=e2b[:, c * SW + q * 512:c * SW + (q + 1) * 512],
                                in_=pe2[:, :])
            else:
                xT = cpool.tile([D, N_LOC], F32)
                nc.sync.dma_start(out=xT[:, :], in_=xT_in[:, :])
                embT = cpool.tile([D, K], F32)
                nc.sync.dma_start(out=embT[:, :], in_=embT_in[:, :])
                sq = cpool.tile([D, K], F32)
                nc.vector.tensor_mul(sq[:, :], embT[:, :], embT[:, :])
                for c in range(K // 512):
                    pe2 = pspool.tile([P, 512], F32, tag="ps")
                    nc.tensor.matmul(out=pe2[:, :], lhsT=negh[:, :],
                                     rhs=sq[:, c * 512:(c + 1) * 512],
                                     start=True, stop=True)
                    nc.scalar.copy(out=e2b[:, c * 512:(c + 1) * 512], in_=pe2[:, :])

            # ---------------- main loop ----------------
            def tile_body(t):
                nsl = slice(t * P, (t + 1) * P)
                sc = spool.tile([P, K], F32, tag="sc")
                for h in range(K // KH):
                    ps = pspool.tile([P, KH], F32, tag="ps")
                    W = mm_width if fp16_split else 512
                    for c in range(KH // W):
                        off = h * KH + c * W
                        out_sl = ps[:, c * W:(c + 1) * W]
                        if fp16_split:
                            nc.tensor.matmul(out=out_sl, lhsT=xs[:, nsl],
                                             rhs=e1d[:, off:off + W],
                                             start=True, stop=False)
                            nc.tensor.matmul(out=out_sl, lhsT=xs[:, nsl],
                                             rhs=e2d[:, off:off + W],
                                             start=False, stop=True)
                        else:
                            nc.tensor.matmul(out=out_sl, lhsT=xT[:, nsl],
                                             rhs=embT[:, off:off + W],
                                             start=True, stop=True)
                    nc.scalar.copy(out=sc[:, h * KH:(h + 1) * KH], in_=ps[:, :])

                junk = jpool.tile([P, K], F32)
                idxf = ipool.tile([P, 1], F32)
                nc.vector._custom_dve(
                    argmax_op, out=junk[:, :], in0=sc[:, :], in1=e2b[:, :],
                    accum_out=idxf[:, :],
                )
                idxi = ipool.tile([P, 1], mybir.dt.int32)
                nc.vector.tensor_copy(out=idxi[:, :], in_=idxf[:, :])

                g = gpool.tile([P, D], F32)
                nc.gpsimd.indirect_dma_start(
                    out=g[:, :], out_offset=None, in_=emb_in[:, :],
                    in_offset=bass.IndirectOffsetOnAxis(ap=idxi[:, :1], axis=0),
                )
                nc.sync.dma_start(out=q_out[t * P:(t + 1) * P, :], in_=g[:, :])

            if r_iters == 1:
                for t in range(NT):
                    tile_body(t)
            else:
                with tc.For_i(0, r_iters, 1):
                    for t in range(NT):
                        tile_body(t)

    nc.compile()
    return nc


def make_in_maps(x: np.ndarray, embed: np.ndarray):
    x = np.ascontiguousarray(x, dtype=np.float32)
    embed = np.ascontiguousarray(embed, dtype=np.float32)
    embT = np.ascontiguousarray(embed.T)
    return [
        {
            "xT": np.ascontiguousarray(x[c * N_LOC:(c + 1) * N_LOC].T),
            "embT": embT,
            "embed": embed,
        }
        for c in range(N_CORES)
    ]


_CACHED_NC = None


def kernel(x: np.ndarray, embed: np.ndarray) -> np.ndarray:
    global _CACHED_NC
    assert x.shape == (N_FULL, D) and embed.shape == (K, D), (
        f"hardcoded for x[{N_FULL},{D}], embed[{K},{D}]; got {x.shape}, {embed.shape}"
    )
    if _CACHED_NC is None:
        _CACHED_NC = build()
    res = run_bass_kernel_spmd(
        _CACHED_NC, make_in_maps(x, embed), core_ids=list(range(N_CORES))
    )
    return np.concatenate([r["q"] for r in res.results], axis=0)
